# revision 17
# baseline (speedup 1.0000x reference)
"""Chamfer distance kernel for Trainium2, 8 NeuronCores.

Strategy
--------
Data-parallel over the batch dim: one batch per core (B=8, n_cores=8).

Per core, the full 8192x8192 squared-distance matrix is generated on the
TensorEngine via an augmented matmul.  We compute e = -d:

    e[n, m] = 2*x1[n].x2[m] - |x1[n]|^2 - |x2[m]|^2 = -d[n, m]

so both outputs are max-reductions (dist = relu(-max e)).  The dot product
is expressed as a K=13 contraction of fp16 "augmented" vectors built on the
host with an fp16 hi/lo split of each coordinate (products of fp16 values
are exact in the fp32 PSUM accumulation, so e matches the fp32 reference
expansion to ~1e-6).

Aug rows (lhs side for x1, rhs side for x2):
    0-2 : 2*hi1_c      <->  hi2_c          (c = x, y, z)
    3-5 : 2*lo1_c      <->  hi2_c
    6-8 : 2*hi1_c      <->  lo2_c
    9,10: -sq1_hi/lo   <->  1
    11,12: 1           <->  -sq2_hi/lo
(rows 13-15 zero padding; K=16)

Device loop, per 128-row block (64 blocks):
    16 matmuls [K=16,128] x [K=16,512] -> PSUM (4 quads of 2048 = 4 banks)
    ScalarE copies each PSUM quad -> SBUF fp16 tile `et` [128, 8192]
    VectorE: colacc = max(colacc, et)            (tensor_tensor, 2x_1P mode)
    VectorE: rowmax[:, i] = max-reduce(et)       (tensor_scalar w/ accum_out,
                                                  4x_2P mode)
Final small reductions (relu(-max)) happen on the host on 2.1 MB/core of
partial results.
"""

import numpy as np

_B, _N, _M = 8, 8192, 8192
_KAUG = 16
_NEGINF = -60000.0

_cache = {}


def _build_nc(n, m, reps=1):
    """Build the per-core Bass program (SPMD, identical on all cores)."""
    import concourse.bass as bass
    import concourse.tile as tile
    from concourse import mybir

    f16, f32 = mybir.dt.float16, mybir.dt.float32
    mx = mybir.AluOpType.max

    assert n % 128 == 0 and m % 512 == 0
    rb = n // 128            # number of 128-row blocks
    qw = min(2048, m)        # PSUM quad width (4 banks of 512 fp32)
    nq = m // qw             # quads per row block
    mmq = qw // 512          # matmuls per quad

    nc = bass.Bass()
    # one combined input tensor -> one DMA -> one producer semaphore for all
    # matmuls (several distinct waits on one Matmult overflow walrus's
    # sync-wait slots)
    augs = nc.dram_tensor("augs", [_KAUG, n + m], f16, kind="ExternalInput")
    rowmax_d = nc.dram_tensor("rowmax", [128, rb], f32, kind="ExternalOutput")
    colmax_d = nc.dram_tensor("colmax", [128, m], f16, kind="ExternalOutput")

    with tile.TileContext(nc) as tc:
        with (
            tc.tile_pool(name="const", bufs=1) as constp,
            tc.tile_pool(name="ets", bufs=2) as etp,
            tc.tile_pool(name="psum", bufs=2, space="PSUM") as psp,
            tc.tile_pool(name="accs", bufs=1) as accp,
        ):
            augs_s = constp.tile([_KAUG, n + m], f16)
            nc.sync.dma_start(augs_s[:], augs[:])
            aug1_s = augs_s[:, 0:n]
            aug2_s = augs_s[:, n:n + m]

            colacc = accp.tile([128, m], f16)
            scratch = accp.tile([128, m], f16)
            rowmaxb = accp.tile([128, rb], f32)

            for r in range(reps):
                for i in range(rb):
                    et = etp.tile([128, m], f16, tag="et")
                    lhsT = aug1_s[:, i * 128:(i + 1) * 128]
                    for q in range(nq):
                        ps = psp.tile([128, qw], f32, tag="ps")
                        for jj in range(mmq):
                            j = q * mmq + jj
                            nc.tensor.matmul(
                                ps[:, jj * 512:(jj + 1) * 512],
                                lhsT,
                                aug2_s[:, j * 512:(j + 1) * 512],
                                start=True,
                                stop=True,
                            )
                        # drain PSUM quad -> SBUF fp16 (ScalarE, own port)
                        nc.scalar.copy(et[:, q * qw:(q + 1) * qw], ps[:])
                    # column partial max (per-partition lanes), DVE 2x_1P
                    if i == 0:
                        nc.vector.tensor_copy(colacc[:], et[:])
                    else:
                        nc.vector.tensor_tensor(colacc[:], colacc[:], et[:], mx)
                    # row max via fused reduce (DVE 4x_2P tensor_scalar)
                    nc.vector.tensor_scalar(
                        scratch[:], et[:], _NEGINF, None,
                        op0=mx, op1=mx,
                        accum_out=rowmaxb[:, i:i + 1],
                    )

            nc.sync.dma_start(rowmax_d[:], rowmaxb[:])
            nc.sync.dma_start(colmax_d[:], colacc[:])

    _elide_redundant_mm_waits(nc)
    _split_multiwait_insts(nc)
    nc.finalize()
    return nc


def _split_multiwait_insts(nc):
    """Walrus allows one sync-wait per instruction; split extras onto
    preceding same-engine NOPs (sequencers execute in order, so a NOP chain
    carrying the waits is equivalent)."""
    from concourse import mybir

    for f in nc.m.functions:
        for bb in f.blocks:
            new_list = []
            for inst in bb.instructions:
                si = getattr(inst, "sync_info", None)
                if si is not None and si.on_wait and len(si.on_wait) > 1:
                    waits = list(si.on_wait)
                    for w in waits[:-1]:
                        nop = mybir.InstNoOp(
                            name=f"I-{nc.next_id()}", ins=[], outs=[]
                        )
                        nop.engine = inst.engine
                        nop.sync_info = mybir.SyncInfo(
                            on_wait=[w], on_update=[]
                        )
                        nc.register_instruction(nop)
                        new_list.append(nop)
                    si.on_wait[:] = [waits[-1]]
                new_list.append(inst)
            bb.instructions[:] = new_list


def _elide_redundant_mm_waits(nc):
    """Drop transitively-implied waits from Matmult instructions.

    Walrus's MM struct holds a single sync-wait, but Tile emits e.g.
    (ACT >= k, PE >= v) on PSUM-bank-reuse matmuls: the PE WAW wait is
    already implied by the ACT WAR wait (the ACT copy that does the k-th
    ACT-sem inc itself waited on PE >= v before reading the bank).  Tile's
    sem assignment is documented as not transitively minimal, so prune here:
    a wait (S >= v) on instruction X is redundant if another wait
    (S' >= k) on X names a producer instruction I_k (the one whose
    completion brings S' to >= k) with its own wait (S >= v') where
    v' >= v.
    """
    from concourse import mybir

    blocks = [bb for f in nc.m.functions for bb in f.blocks]
    # ordered inc events per semaphore id: list of (cumulative_value, inst)
    incs = {}
    for bb in blocks:
        for inst in bb.instructions:
            si = getattr(inst, "sync_info", None)
            if si is None:
                continue
            for up in si.on_update or []:
                if up.sync_type == "semaphore" and up.update_mode == "sem-inc":
                    lst = incs.setdefault(up.id, [])
                    prev = lst[-1][0] if lst else 0
                    lst.append((prev + (up.update_value or 1), inst))

    def producer_of(sem_id, value):
        for cum, inst in incs.get(sem_id, []):
            if cum >= value:
                return inst
        return None

    leftover = []
    for bb in blocks:
        for inst in bb.instructions:
            si = getattr(inst, "sync_info", None)
            if si is None or not si.on_wait or len(si.on_wait) < 2:
                continue
            waits = list(si.on_wait)
            kept = list(waits)
            for w in waits:
                if w.wait_mode != "sem-ge-imm":
                    continue
                others = [o for o in kept if o is not w]
                for o in others:
                    if o.wait_mode != "sem-ge-imm":
                        continue
                    prod = producer_of(o.id, o.wait_value)
                    psi = getattr(prod, "sync_info", None) if prod else None
                    if psi is None:
                        continue
                    if any(
                        pw.sync_type == "semaphore"
                        and pw.id == w.id
                        and pw.wait_mode == "sem-ge-imm"
                        and pw.wait_value >= w.wait_value
                        for pw in psi.on_wait or []
                    ):
                        kept.remove(w)
                        break
            if len(kept) != len(waits):
                si.on_wait[:] = kept
            if len(kept) >= 2:
                leftover.append((inst.name, type(inst).__name__, list(kept)))
    if leftover:
        print(f"[kernel] WARNING: {len(leftover)} instructions still have "
              f">=2 sync waits, e.g. {leftover[:3]}")


def _get_nc(n=_N, m=_M, reps=1):
    key = (n, m, reps)
    if key not in _cache:
        _cache[key] = _build_nc(n, m, reps)
    return _cache[key]


def _split16(v):
    hi = v.astype(np.float16)
    lo = (v - hi.astype(np.float32)).astype(np.float16)
    return hi, lo


def build_augs(x1, x2):
    """Host-side prep: [n,3]/[m,3] fp32 -> fp16 augmented K-vectors."""
    n, m = x1.shape[0], x2.shape[0]
    h1, l1 = _split16(x1)
    l1 = l1.astype(np.float16)
    h2, l2 = _split16(x2)
    sq1 = np.einsum("nc,nc->n", x1, x1, dtype=np.float32)
    sq2 = np.einsum("mc,mc->m", x2, x2, dtype=np.float32)
    s1h, s1l = _split16(sq1)
    s2h, s2l = _split16(sq2)

    a1 = np.zeros((_KAUG, n), np.float16)
    a2 = np.zeros((_KAUG, m), np.float16)
    a1[0:3] = (h1.T * np.float16(2))
    a2[0:3] = h2.T
    a1[3:6] = (l1.T * np.float16(2))
    a2[3:6] = h2.T
    a1[6:9] = (h1.T * np.float16(2))
    a2[6:9] = l2.T
    a1[9] = -s1h
    a1[10] = -s1l
    a2[9] = 1
    a2[10] = 1
    a1[11] = 1
    a1[12] = 1
    a2[11] = -s2h
    a2[12] = -s2l
    return a1, a2


def _postprocess(res_list, n, m):
    b = len(res_list)
    dist1 = np.empty((b, n), np.float32)
    dist2 = np.empty((b, m), np.float32)
    for c, r in enumerate(res_list):
        rm = np.asarray(r["rowmax"], np.float32)          # [128, rb]
        cm = np.asarray(r["colmax"], np.float32)          # [128, m]
        dist1[c] = np.maximum(-rm.T.reshape(-1), 0.0)     # global n = i*128+p
        dist2[c] = np.maximum(-cm.max(axis=0), 0.0)
    return dist1, dist2


def kernel(xyz1, xyz2):
    from concourse.bass_utils import run_bass_kernel_spmd

    xyz1 = np.asarray(xyz1, np.float32)
    xyz2 = np.asarray(xyz2, np.float32)
    b, n, _ = xyz1.shape
    m = xyz2.shape[1]

    nc = _get_nc(n, m)
    in_maps = []
    for i in range(b):
        a1, a2 = build_augs(xyz1[i], xyz2[i])
        in_maps.append({"augs": np.concatenate([a1, a2], axis=1)})

    res = run_bass_kernel_spmd(nc, in_maps, core_ids=list(range(b)))
    return _postprocess(res.results, n, m)
